# revision 1
# baseline (speedup 1.0000x reference)
"""Distributed 2-layer GCN + mean-pool + linear + sigmoid on 8 TRN2 NeuronCores.

Sharding: nodes (and their in-edges) are sharded across 8 cores by contiguous
dst ranges; weights replicated.  h1 is exchanged with a single fp8 AllGather
(the only fast collective on this stack: shared-output RDMA writes; AllToAll/
ReduceScatter carry ~200-430us fixed cost), then compacted per-producer into
an int16-addressable table (stage A).  Pooled partial sums are exchanged with
a small bf16 AllGather and combined at compile-time-known graph offsets.

Aggregation (aggregate-then-transform; GCN is linear so this is exact): for
each 128-node window, gather the x/h rows of in-edge sources (int16
dma_gather from host-compacted tables), build a sparse scatter matrix
S[e, n] = norm_e * onehot(dstloc_e) on DVE (iota + fused is_equal/mult), and
accumulate aggT[d, n] += Xg^T @ S on the TensorEngine.  Self-loops are a
per-window diagonal matmul; bias is a K=1 ones-row matmul into the same PSUM
accumulation; relu (and the 1/cnt pooling scale in layer 2) is one fused
activation op.

Device-time reductions vs the first working version (sim: 1107us -> 653us):
fp8 h1 exchange + layer-2 gather/aggregation (26MB AllGather instead of
52MB); metadata tables fed pre-transposed from host (contiguous DMA); xp and
h1 SBUF-resident (no per-window reloads); h1 DRAM writes batched 4 windows
per DMA; deep tile pools for cross-window pipelining; bf16 pool exchange
with tightened mask width; single-DMA final output.

Compact-id space: for each (producer p, consumer c) pair the unique source
slots are ranked (sorted by slot); compact id = p*MAXH + rank.  The same id
space indexes xc (x features, layer 1) and h_compact (stage-A output,
layer 2), so one edge-index table serves both layers.
"""

import math
from contextlib import ExitStack

import numpy as np

P = 128
IN_DIM = 128
HID = 256
BF16 = True  # bf16 matmul operands / gather tables (PSUM accum stays fp32)
EDT_FP8 = True   # h1 exchange (AllGather + stage A + L2 gather) in fp8e4
XC_FP8 = False   # xc table + L1 edge gather in fp8e4
POOL_BF16 = True  # pool partial exchange in bf16


def XDT_MYBIR():
    import concourse.mybir as mybir
    return mybir.dt.float8e4 if XC_FP8 else (
        mybir.dt.bfloat16 if BF16 else mybir.dt.float32)
ABLATE = set()  # timing ablations: subset of {"h1ag", "poolag", "stagea"}


def EDT_MYBIR():
    import concourse.mybir as mybir
    return mybir.dt.float8e4 if EDT_FP8 else (
        mybir.dt.bfloat16 if BF16 else mybir.dt.float32)
SACALL = 1024  # max gather call size (64 desc/engine packet limit)
N_CORES = 8


def _sa_calls(SAK):
    """Call plan: sizes (each %128==0, <=1024) with offsets."""
    plan, off = [], 0
    while off < SAK:
        L = min(SACALL, SAK - off)
        plan.append((off, L))
        off += L
    return plan


# ----------------------------------------------------------------- host prep


def _pack(node_ids, degs, n_bins):
    """Least-loaded-first packing into n_bins bins of <=128 nodes, balancing
    sum of degs.  Returns (win, pos, max_load)."""
    import heapq

    order = np.argsort(-degs, kind="stable")
    nb = len(node_ids)
    win = np.empty(nb, np.int32)
    pos = np.empty(nb, np.int32)
    counts = np.zeros(n_bins, np.int32)
    loads = np.zeros(n_bins, np.int64)
    heap = [(0, b) for b in range(n_bins)]
    heapq.heapify(heap)
    for oi in order:
        while True:
            load, b = heapq.heappop(heap)
            if counts[b] < P:
                break
        win[oi] = b
        pos[oi] = counts[b]
        counts[b] += 1
        loads[b] = load + degs[oi]
        heapq.heappush(heap, (loads[b], b))
    return win, pos, loads.max()


def wrap16_plan(vals, plan):
    """Layout int16 idx array for dma_gather: per call segment of size L,
    idx j lands at [j%16, col0 + j//16].  Replicated across 8 gpsimd cores."""
    v = np.asarray(vals, np.int16)
    segs = []
    per = plan[-1][0] + plan[-1][1]
    n = len(v) // per
    for r in range(n):
        for off, L in plan:
            seg = v[r * per + off : r * per + off + L]
            segs.append(seg.reshape(L // 16, 16).T)
    out = np.concatenate(segs, axis=1)
    return np.tile(out, (8, 1))


def _prep(x, edge_index, batch, n_graphs, n_cores):
    N = x.shape[0]
    E = edge_index.shape[1]
    NPC = N // n_cores

    src = np.asarray(edge_index[0], np.int64)
    dst = np.asarray(edge_index[1], np.int64)
    batch = np.asarray(batch, np.int64)

    deg = (np.bincount(dst, minlength=N) + 1).astype(np.float32)
    dinv = (1.0 / np.sqrt(deg)).astype(np.float32)
    norm_e = (dinv[src] * dinv[dst]).astype(np.float32)
    selfw = (dinv * dinv).astype(np.float32)
    cnt = np.bincount(batch, minlength=n_graphs).astype(np.float32)
    icnt_g = (1.0 / np.maximum(cnt, 1.0)).astype(np.float32)

    edge_core = dst // NPC  # consumer (dst owner)
    src_core = src // NPC  # producer (src owner)

    # pass 1: per-core window packing of own nodes.
    indeg = np.bincount(dst, minlength=N)
    W_base = math.ceil(NPC / P)
    chosen = None
    for K_try, extra in [(3, 0), (3, 2), (3, 4), (3, 6), (4, 0)]:
        W_try = W_base + extra
        cap = K_try * P
        packs = []
        ok = True
        for c in range(n_cores):
            ids = np.arange(c * NPC, (c + 1) * NPC)
            w, p, mx = _pack(ids, indeg[ids].astype(np.int64), W_try)
            if mx > cap:
                ok = False
                break
            packs.append((w, p))
        if ok:
            chosen = (K_try, W_try, packs)
            break
    assert chosen is not None
    K, W, packs = chosen
    NSLOT = W * P
    win_of = np.empty(N, np.int32)
    pos_of = np.empty(N, np.int32)
    for c in range(n_cores):
        ids = np.arange(c * NPC, (c + 1) * NPC)
        w, p = packs[c]
        win_of[ids] = w
        pos_of[ids] = p
    slot_of = win_of.astype(np.int64) * P + pos_of  # local slot within owner
    NCHUNK = W * K
    ES = NCHUNK * P

    # slot -> node id per core
    inv_slot = np.full((n_cores, NSLOT), -1, np.int64)
    for c in range(n_cores):
        ids = np.arange(c * NPC, (c + 1) * NPC)
        inv_slot[c, slot_of[ids]] = ids

    # unique src slots per (producer, consumer) pair
    u_pc = [[None] * n_cores for _ in range(n_cores)]  # [p][c] -> slots
    maxcnt = 0
    for c in range(n_cores):
        m = edge_core == c
        for p in range(n_cores):
            mp = m & (src_core == p)
            u = np.unique(slot_of[src[mp]])
            u_pc[p][c] = u
            maxcnt = max(maxcnt, len(u))
    MAXH = int(np.ceil(maxcnt / P)) * P
    UC = n_cores * MAXH
    assert UC <= 32768, f"compact table too large: {UC} (maxcnt {maxcnt})"

    # gather batches are capped at 1024 idxs
    NBW = 1
    for cand in (4, 2):
        if W % cand == 0 and cand * K * 128 <= 1024:
            NBW = cand
            break
    if NBW == 1 and K * 128 > 1024:
        raise AssertionError(f"K={K} too large for single gather batch")

    GBLK = n_graphs // n_cores  # graphs per core for ReduceScatter

    per_core = []
    import ml_dtypes

    cdt = ml_dtypes.bfloat16 if BF16 else np.float32
    xnp = np.asarray(x)

    for c in range(n_cores):
        m = np.flatnonzero(edge_core == c)
        e_src, e_dst, e_norm = src[m], dst[m], norm_e[m]
        e_win = win_of[e_dst]
        order = np.argsort(e_win, kind="stable")
        e_src, e_dst, e_norm, e_win = (
            e_src[order],
            e_dst[order],
            e_norm[order],
            e_win[order],
        )
        wc = np.bincount(e_win, minlength=W)
        assert wc.max() <= K * P

        # compact id per edge source: p*MAXH + rank in u_pc[p][c]
        ep = e_src // NPC
        cids = np.empty(len(e_src), np.int64)
        for p in range(n_cores):
            mk = ep == p
            cids[mk] = p * MAXH + np.searchsorted(
                u_pc[p][c], slot_of[e_src[mk]]
            )

        # xc table: x features in compact layout (for layer 1)
        import ml_dtypes as _md
        xdt_np = _md.float8_e4m3 if XC_FP8 else cdt
        xc = np.zeros((UC, IN_DIM), xdt_np)
        for p in range(n_cores):
            u = u_pc[p][c]
            xc[p * MAXH : p * MAXH + len(u)] = xnp[inv_slot[p, u]]

        # stage-A index table (this core as consumer): for each producer p,
        # local slots of u_pc[p][c] within p's h_full block, padded with 0
        # (gathers row 0 into unused compact rows; negative "skip" indices
        # crash the DGE on this stack).
        sidx = np.zeros(UC, np.int64)
        for p in range(n_cores):
            u = u_pc[p][c]
            sidx[p * MAXH : p * MAXH + len(u)] = u

        # per-window edge slots
        gidx_slots = np.zeros(ES, np.int64)
        dstloc = np.zeros(ES, np.float32)
        nrm = np.zeros(ES, np.float32)
        off = np.concatenate([[0], np.cumsum(wc)])
        for w in range(W):
            a, b = off[w], off[w + 1]
            sl = w * K * P
            nw = b - a
            gidx_slots[sl : sl + nw] = cids[a:b]
            dstloc[sl : sl + nw] = pos_of[e_dst[a:b]]
            nrm[sl : sl + nw] = e_norm[a:b]

        # dn table pre-transposed host-side: [P, 2*NCHUNK]
        dn = np.concatenate(
            [dstloc.reshape(NCHUNK, P).T, nrm.reshape(NCHUNK, P).T], axis=1
        ).astype(np.float32)

        # per-slot node metadata (window-permuted own nodes), [P, W] layouts
        ids = np.arange(c * NPC, (c + 1) * NPC)
        slot_node = np.full(NSLOT, -1, np.int64)
        slot_node[slot_of[ids]] = ids
        valid = slot_node >= 0
        dsq = np.zeros(NSLOT, np.float32)
        dsq[valid] = selfw[slot_node[valid]]
        xp = np.zeros((NSLOT, IN_DIM), cdt)
        xp[valid] = xnp[slot_node[valid]]
        xp = np.ascontiguousarray(
            xp.reshape(W, P, IN_DIM).transpose(1, 0, 2).reshape(P, W * IN_DIM))
        bvals = np.full(NSLOT, -1.0, np.float32)
        icn = np.zeros(NSLOT, np.float32)
        gb_c = int(batch[c * NPC])
        bvals[valid] = (batch[slot_node[valid]] - gb_c).astype(np.float32)
        icn[valid] = icnt_g[batch[slot_node[valid]]]

        per_core.append(
            dict(
                xc=xc,
                xp=xp,
                _gidx_slots=gidx_slots,
                dn=dn,
                dsq=dsq.reshape(W, P).T.astype(np.float32),
                bloc=bvals.reshape(W, P).T.astype(np.float32),
                icnt=icn.reshape(W, P).T.astype(np.float32),
                sidx=wrap16_plan(sidx, _sa_calls(MAXH)),
                gb=gb_c,
            )
        )

    gbs = [pc["gb"] for pc in per_core]
    gspan = max(
        int(batch[(c + 1) * NPC - 1]) - gbs[c] + 1 for c in range(n_cores)
    )
    GW = int(np.ceil(gspan / 16)) * 16
    assert GW <= 512, GW

    meta = dict(
        N=N, E=E, NPC=NPC, W=W, NSLOT=NSLOT, K=K, NCHUNK=NCHUNK, ES=ES,
        MAXH=MAXH, UC=UC, GW=GW, gbs=gbs, n_graphs=n_graphs,
        n_cores=n_cores, NBW=NBW, GBLK=GBLK,
    )
    return per_core, meta


# ------------------------------------------------------------- device kernel


def _build_program(meta):
    import concourse.bacc as bacc
    import concourse.bass as bass
    import concourse.mybir as mybir
    import concourse.tile as tile

    f32 = mybir.dt.float32
    cdt = mybir.dt.bfloat16 if BF16 else mybir.dt.float32
    edt = EDT_MYBIR()
    xdt = XDT_MYBIR()
    pdt = mybir.dt.bfloat16 if POOL_BF16 else f32
    i16 = mybir.dt.int16
    i32 = mybir.dt.int32
    Alu = mybir.AluOpType
    Act = mybir.ActivationFunctionType

    W, K, ES, NSLOT = meta["W"], meta["K"], meta["ES"], meta["NSLOT"]
    NCHUNK, MAXH, UC, GW = meta["NCHUNK"], meta["MAXH"], meta["UC"], meta["GW"]
    gbs = meta["gbs"]
    G = meta["n_graphs"]
    GBLK = meta["GBLK"]
    n_cores = meta["n_cores"]
    GLOBW = max(G, max(gbs) + GW)
    NBW = meta["NBW"]  # windows per gather batch
    EB = NBW * K * P  # edge slots per gather batch
    HB = 4  # windows per h1_local write batch

    nc = bacc.Bacc(None, target_bir_lowering=False)

    ext_in = {}
    for name, shape, dt in [
        ("xc", [UC, IN_DIM], xdt),
        ("xp", [P, W * IN_DIM], cdt),
        ("gidx", [P, ES // 16], i16),
        ("dn", [P, 2 * NCHUNK], f32),
        ("dsq", [P, W], f32),
        ("bloc", [P, W], f32),
        ("icnt", [P, W], f32),
        ("sidx", [P, UC // 16], i16),
        ("w1", [IN_DIM, HID], cdt),
        ("w2", [HID, HID], cdt),
        ("wf", [HID, 1], f32),
        ("b1", [1, HID], cdt),
        ("b2", [1, HID], cdt),
        ("bf", [1, 1], f32),
    ]:
        ext_in[name] = nc.dram_tensor(name, shape, dt, kind="ExternalInput")
    out_ext = nc.dram_tensor("out", [G, 1], f32, kind="ExternalOutput")

    h1_local = nc.dram_tensor("h1_local", [NSLOT, HID], edt)
    h_full = nc.dram_tensor("h_full", [NSLOT * n_cores, HID], edt,
                            addr_space="Shared")
    h_compact = nc.dram_tensor("h_compact", [UC, HID], edt)
    pool_part = nc.dram_tensor("pool_part", [HID, GW], pdt)
    pool_all = nc.dram_tensor("pool_all", [HID * n_cores, GW], pdt,
                              addr_space="Shared")

    core_ids = list(range(n_cores))

    with ExitStack() as ctx:
        tc = ctx.enter_context(tile.TileContext(nc, num_cores=n_cores))
        cst = ctx.enter_context(tc.tile_pool(name="cst", bufs=1))
        sbw = ctx.enter_context(tc.tile_pool(name="sbw", bufs=10))
        xgp = ctx.enter_context(tc.tile_pool(name="xgp", bufs=4))
        hp = ctx.enter_context(tc.tile_pool(name="hp", bufs=6))
        ps_agg = ctx.enter_context(
            tc.tile_pool(name="ps_agg", bufs=4, space="PSUM"))
        ps_tr = ctx.enter_context(
            tc.tile_pool(name="ps_tr", bufs=2, space="PSUM"))
        ps_pool = ctx.enter_context(
            tc.tile_pool(name="ps_pool", bufs=1, space="PSUM"))

        # ---- constants / metadata loads (all contiguous layouts)
        gidx_t = cst.tile([P, ES // 16], i16)
        nc.sync.dma_start(out=gidx_t[:], in_=ext_in["gidx"][:, :])
        sidx_t = cst.tile([P, UC // 16], i16)
        nc.sync.dma_start(out=sidx_t[:], in_=ext_in["sidx"][:, :])
        dn_t = cst.tile([P, 2 * NCHUNK], f32)
        nc.sync.dma_start(out=dn_t[:], in_=ext_in["dn"][:, :])
        dsq_t = cst.tile([P, W], f32)
        nc.sync.dma_start(out=dsq_t[:], in_=ext_in["dsq"][:, :])
        bloc_t = cst.tile([P, W], f32)
        nc.sync.dma_start(out=bloc_t[:], in_=ext_in["bloc"][:, :])
        icnt_t = cst.tile([P, W], f32)
        nc.sync.dma_start(out=icnt_t[:], in_=ext_in["icnt"][:, :])
        w1_t = cst.tile([IN_DIM, HID], cdt)
        nc.sync.dma_start(out=w1_t[:], in_=ext_in["w1"][:, :])
        w2_t = cst.tile([P, 2 * HID], cdt)  # W2 K-halves side by side
        nc.sync.dma_start(
            out=w2_t[:].rearrange("k (s h) -> k s h", s=2),
            in_=ext_in["w2"].rearrange("(s k) h -> k s h", k=P))
        wf_t = cst.tile([P, 2], f32)  # wf halves: [:, 0], [:, 1]
        nc.sync.dma_start(
            out=wf_t[:].rearrange("k (s o) -> k s o", s=2),
            in_=ext_in["wf"].rearrange("(s k) o -> k s o", k=P))
        b1_t = cst.tile([1, HID], cdt)
        nc.sync.dma_start(out=b1_t[:], in_=ext_in["b1"][:, :])
        b2_t = cst.tile([1, HID], cdt)
        nc.sync.dma_start(out=b2_t[:], in_=ext_in["b2"][:, :])
        bf_t = cst.tile([1, 1], f32)
        nc.sync.dma_start(out=bf_t[:], in_=ext_in["bf"][:, :])

        # xp resident in SBUF: [P, W, IN_DIM] (host feeds pre-transposed)
        xp_all = cst.tile([P, W, IN_DIM], cdt)
        nc.sync.dma_start(
            out=xp_all[:],
            in_=ext_in["xp"].rearrange("p (w d) -> p w d", d=IN_DIM))

        # h1 resident in SBUF (written by L1, read by L2 self-loop);
        # kept in the exchange dtype so DRAM writes need no cast.
        h1_all = cst.tile([P, W, HID], edt)

        ones_t = cst.tile([1, P], cdt)
        nc.vector.memset(ones_t[:], 1.0)
        ones_f = cst.tile([1, P], f32)
        nc.vector.memset(ones_f[:], 1.0)

        iota_i = cst.tile([P, K * P], i32)
        nc.gpsimd.iota(iota_i[:], pattern=[[0, K], [1, P]], base=0,
                       channel_multiplier=0)
        iota_f = cst.tile([P, K * P], f32)
        nc.vector.tensor_copy(out=iota_f[:], in_=iota_i[:])
        iotag_i = cst.tile([P, GW], i32)
        nc.gpsimd.iota(iotag_i[:], pattern=[[1, GW]], base=0,
                       channel_multiplier=0)
        iotag_f = cst.tile([P, GW], f32)
        nc.vector.tensor_copy(out=iotag_f[:], in_=iotag_i[:])
        pcol_i = cst.tile([P, 1], i32)
        nc.gpsimd.iota(pcol_i[:], pattern=[[0, 1]], base=0,
                       channel_multiplier=1)
        pcol_f = cst.tile([P, 1], f32)
        nc.vector.tensor_copy(out=pcol_f[:], in_=pcol_i[:])

        # ---------------- layer 1
        next_w = 0
        for b in range(W // NBW):
            xg = xgp.tile([P, NBW * K, IN_DIM], xdt, tag="xg")
            nc.gpsimd.dma_gather(
                out_ap=xg[:],
                in_ap=ext_in["xc"][:, :],
                idxs_ap=gidx_t[:, b * (EB // 16) : (b + 1) * (EB // 16)],
                num_idxs=EB,
                num_idxs_reg=EB,
                elem_size=IN_DIM,
            )
            for wl in range(NBW):
                w = b * NBW + wl
                aggp = ps_agg.tile([P, P], f32, space="PSUM", tag="aggp")
                for ck in range(K):
                    s_t = sbw.tile([P, P], cdt, tag="s_t")
                    cg = w * K + ck
                    nc.vector.tensor_scalar(
                        out=s_t[:],
                        in0=iota_f[:, ck * P : (ck + 1) * P],
                        scalar1=dn_t[:, cg : cg + 1],
                        scalar2=dn_t[:, NCHUNK + cg : NCHUNK + cg + 1],
                        op0=Alu.is_equal,
                        op1=Alu.mult,
                    )
                    nc.tensor.matmul(
                        out=aggp[:],
                        lhsT=xg[:, wl * K + ck, :],
                        rhs=s_t[:],
                        start=(ck == 0),
                        stop=False,
                    )
                # self loops: aggT += xp_w^T @ diag(dsq_w)
                diag = sbw.tile([P, P], cdt, tag="diag")
                nc.vector.tensor_scalar(
                    out=diag[:],
                    in0=iota_f[:, 0:P],
                    scalar1=pcol_f[:, 0:1],
                    scalar2=dsq_t[:, w : w + 1],
                    op0=Alu.is_equal,
                    op1=Alu.mult,
                )
                nc.tensor.matmul(
                    out=aggp[:], lhsT=xp_all[:, w, :], rhs=diag[:],
                    start=False, stop=True)
                agg_sb = sbw.tile([P, P], cdt, tag="agg_sb")
                nc.scalar.copy(out=agg_sb[:], in_=aggp[:])
                # transform: h1 = relu(aggT.T @ W1 + b1)
                hpsum = ps_tr.tile([P, HID], f32, space="PSUM", tag="hpsum")
                nc.tensor.matmul(
                    out=hpsum[:], lhsT=agg_sb[:], rhs=w1_t[:],
                    start=True, stop=False)
                nc.tensor.matmul(
                    out=hpsum[:], lhsT=ones_t[:], rhs=b1_t[:],
                    start=False, stop=True)
                nc.scalar.activation(
                    out=h1_all[:, w, :], in_=hpsum[:], func=Act.Relu)
            # batched h1_local writes every HB windows (tail handled)
            done = (b + 1) * NBW
            while next_w + HB <= done or (done == W and next_w < W):
                nb_ = min(HB, W - next_w)
                nc.sync.dma_start(
                    out=h1_local[next_w * P : (next_w + nb_) * P, :].rearrange(
                        "(c p) d -> p c d", p=P),
                    in_=h1_all[:, next_w : next_w + nb_, :])
                next_w += nb_

        # ---------------- AllGather h1, then per-producer compaction
        if "h1ag" not in ABLATE:
            nc.gpsimd.collective_compute(
                "AllGather", Alu.bypass, replica_groups=[core_ids],
                ins=[h1_local[:, :]], outs=[h_full[:, :]])
        sa_plan = [] if "stagea" in ABLATE else _sa_calls(MAXH)
        for p in range(n_cores):
            for off, L in sa_plan:
                hc = xgp.tile([P, SACALL // P, HID], edt, tag="hc")
                cbase = p * MAXH + off
                icol = cbase // 16
                nc.gpsimd.dma_gather(
                    out_ap=hc[:, : L // P, :],
                    in_ap=h_full[p * NSLOT : (p + 1) * NSLOT, :],
                    idxs_ap=sidx_t[:, icol : icol + L // 16],
                    num_idxs=L,
                    num_idxs_reg=L,
                    elem_size=HID,
                )
                nc.sync.dma_start(
                    out=h_compact[cbase : cbase + L, :].rearrange(
                        "(c p) d -> p c d", p=P),
                    in_=hc[:, : L // P, :])

        # ---------------- layer 2 + pooling
        # glob: [2][P, GLOBW] f32 pooled partial sums at global graph columns
        glob = []
        for h in range(2):
            gt = cst.tile([P, GLOBW], f32, tag=f"glob{h}")
            nc.vector.memset(gt[:], 0.0)
            glob.append(gt)
        poolp = []
        for h in range(2):
            pt = ps_pool.tile([P, GW], f32, space="PSUM", tag=f"poolp{h}")
            poolp.append(pt)
        for b in range(W // NBW):
            hg = xgp.tile([P, NBW * K, HID], edt, tag="hg")
            nc.gpsimd.dma_gather(
                out_ap=hg[:],
                in_ap=h_compact[:, :],
                idxs_ap=gidx_t[:, b * (EB // 16) : (b + 1) * (EB // 16)],
                num_idxs=EB,
                num_idxs_reg=EB,
                elem_size=HID,
            )
            for wl in range(NBW):
                w = b * NBW + wl
                aggp2 = []
                for h in range(2):
                    a2t = ps_agg.tile([P, P], f32, space="PSUM", tag="aggp")
                    aggp2.append(a2t)
                for ck in range(K):
                    s_t = sbw.tile([P, P], edt, tag="s_t2")
                    cg = w * K + ck
                    nc.vector.tensor_scalar(
                        out=s_t[:],
                        in0=iota_f[:, ck * P : (ck + 1) * P],
                        scalar1=dn_t[:, cg : cg + 1],
                        scalar2=dn_t[:, NCHUNK + cg : NCHUNK + cg + 1],
                        op0=Alu.is_equal,
                        op1=Alu.mult,
                    )
                    for h in range(2):
                        nc.tensor.matmul(
                            out=aggp2[h][:],
                            lhsT=hg[:, wl * K + ck, h * P : (h + 1) * P],
                            rhs=s_t[:],
                            start=(ck == 0),
                            stop=False,
                        )
                diag = sbw.tile([P, P], edt, tag="diag2")
                nc.vector.tensor_scalar(
                    out=diag[:],
                    in0=iota_f[:, 0:P],
                    scalar1=pcol_f[:, 0:1],
                    scalar2=dsq_t[:, w : w + 1],
                    op0=Alu.is_equal,
                    op1=Alu.mult,
                )
                for h in range(2):
                    nc.tensor.matmul(
                        out=aggp2[h][:],
                        lhsT=h1_all[:, w, h * P : (h + 1) * P],
                        rhs=diag[:],
                        start=False,
                        stop=True,
                    )
                hpsum = ps_tr.tile([P, HID], f32, space="PSUM", tag="hpsum")
                for h in range(2):
                    agg_sb = sbw.tile([P, P], cdt, tag=f"agg2sb{h}")
                    nc.scalar.copy(out=agg_sb[:], in_=aggp2[h][:])
                    nc.tensor.matmul(
                        out=hpsum[:],
                        lhsT=agg_sb[:],
                        rhs=w2_t[:, h * HID : (h + 1) * HID],
                        start=(h == 0),
                        stop=False,
                    )
                nc.tensor.matmul(
                    out=hpsum[:], lhsT=ones_t[:], rhs=b2_t[:],
                    start=False, stop=True)
                # h2s = relu(h2) * icnt
                h2s = hp.tile([P, HID], edt, tag="h2s")
                nc.scalar.activation(
                    out=h2s[:], in_=hpsum[:], func=Act.Relu,
                    scale=icnt_t[:, w : w + 1])
                # pooling mask + matmuls
                mask = sbw.tile([P, GW], edt, tag="mask")
                nc.vector.tensor_scalar(
                    out=mask[:], in0=iotag_f[:], scalar1=bloc_t[:, w : w + 1],
                    scalar2=None, op0=Alu.is_equal)
                for h in range(2):
                    nc.tensor.matmul(
                        out=poolp[h][:],
                        lhsT=h2s[:, h * P : (h + 1) * P],
                        rhs=mask[:],
                        start=(w == 0),
                        stop=(w == W - 1),
                    )

        # ---------------- pooled shards out + AllGather
        for h in range(2):
            pp = hp.tile([P, GW], pdt, tag="ppsb")
            nc.scalar.copy(out=pp[:], in_=poolp[h][:])
            nc.sync.dma_start(
                out=pool_part[h * P : (h + 1) * P, :], in_=pp[:])
        if "poolag" not in ABLATE:
            nc.gpsimd.collective_compute(
                "AllGather", Alu.bypass, replica_groups=[core_ids],
                ins=[pool_part[:, :]], outs=[pool_all[:, :]])

        # ---------------- combine shards into global pooledT
        NBLK = math.ceil(G / P)
        for r in range(n_cores):
            for h in range(2):
                sh = hp.tile([P, GW], pdt, tag="sh")
                nc.sync.dma_start(
                    out=sh[:],
                    in_=(pool_part[h * P : (h + 1) * P, :]
                         if "poolag" in ABLATE else
                         pool_all[r * HID + h * P : r * HID + (h + 1) * P, :]))
                nc.vector.tensor_tensor(
                    out=glob[h][:, gbs[r] : gbs[r] + GW],
                    in0=glob[h][:, gbs[r] : gbs[r] + GW],
                    in1=sh[:],
                    op=Alu.add,
                )

        # ---------------- final linear + sigmoid
        out_sb = cst.tile([P, NBLK], f32, tag="out_sb")
        for bk in range(NBLK):
            lp = ps_tr.tile([P, 1], f32, space="PSUM", tag="hpsum")
            for h in range(2):
                nc.tensor.matmul(
                    out=lp[:],
                    lhsT=glob[h][:, bk * P : (bk + 1) * P],
                    rhs=wf_t[:, h : h + 1],
                    start=(h == 0),
                    stop=False,
                )
            nc.tensor.matmul(
                out=lp[:], lhsT=ones_f[:], rhs=bf_t[:],
                start=False, stop=True)
            nc.scalar.activation(
                out=out_sb[:, bk : bk + 1], in_=lp[:], func=Act.Sigmoid)
        assert G == NBLK * P
        nc.sync.dma_start(
            out=out_ext.rearrange("(b p) o -> p (b o)", p=P),
            in_=out_sb[:, :NBLK])

    nc.compile()
    return nc, meta


def _layout_gidx(gidx_slots, meta):
    """edge-slot compact ids -> int16 wrapped layout for batched dma_gather."""
    K = meta["K"]
    EB = meta["NBW"] * K * P
    v = np.asarray(gidx_slots, np.int16).reshape(-1, EB)
    cw = EB // 16
    out = np.empty((16, v.size // 16), np.int16)
    for b in range(v.shape[0]):
        out[:, b * cw : (b + 1) * cw] = v[b].reshape(cw, 16).T
    return np.tile(out, (8, 1))


def kernel(x, edge_index, batch, W1, b1, W2, b2, Wf, bf,
           n_graphs=2048, n_cores=8):
    from concourse.bass_utils import run_bass_kernel_spmd

    x = np.asarray(x, np.float32)
    per_core, meta = _prep(x, edge_index, batch, n_graphs, n_cores)
    nc, meta = _build_program(meta)

    import ml_dtypes
    cdt = ml_dtypes.bfloat16 if BF16 else np.float32
    w_comm = dict(
        w1=np.asarray(W1, np.float32).astype(cdt),
        w2=np.asarray(W2, np.float32).astype(cdt),
        wf=np.asarray(Wf, np.float32).reshape(HID, 1),
        b1=np.asarray(b1, np.float32).reshape(1, HID).astype(cdt),
        b2=np.asarray(b2, np.float32).reshape(1, HID).astype(cdt),
        bf=np.asarray(bf, np.float32).reshape(1, 1),
    )
    in_maps = []
    for pc in per_core:
        m = dict(w_comm)
        m["xc"] = pc["xc"]
        m["xp"] = pc["xp"]
        m["gidx"] = _layout_gidx(pc["_gidx_slots"], meta)
        m["dn"] = pc["dn"]
        m["dsq"] = pc["dsq"]
        m["bloc"] = pc["bloc"]
        m["icnt"] = pc["icnt"]
        m["sidx"] = pc["sidx"]
        in_maps.append(m)

    res = run_bass_kernel_spmd(nc, in_maps, list(range(n_cores)))
    return np.asarray(res.results[0]["out"], np.float32)



# revision 6
# speedup vs baseline: 1.4483x; 1.4483x over previous
"""Distributed 2-layer GCN + mean-pool + linear + sigmoid on 8 TRN2 NeuronCores.

Sharding: nodes (and their in-edges) are sharded across 8 cores by contiguous
dst ranges; weights replicated.  h1 is exchanged with a single fp8 AllGather
(the only fast collective on this stack: shared-output RDMA writes; AllToAll/
ReduceScatter carry ~200-430us fixed cost), then compacted per-producer into
an int16-addressable table (stage A).  Pooled partial sums are exchanged with
a small bf16 AllGather and combined at compile-time-known graph offsets.

Aggregation (aggregate-then-transform; GCN is linear so this is exact): for
each 128-node window, gather the x/h rows of in-edge sources (int16
dma_gather from host-compacted tables), build a sparse scatter matrix
S[e, n] = norm_e * onehot(dstloc_e) on DVE (iota + fused is_equal/mult), and
accumulate aggT[d, n] += Xg^T @ S on the TensorEngine.  Self-loops are a
per-window diagonal matmul; bias is a K=1 ones-row matmul into the same PSUM
accumulation; relu (and the 1/cnt pooling scale in layer 2) is one fused
activation op.

Device-time reductions vs the first working version (sim: 1107us -> 653us):
fp8 h1 exchange + layer-2 gather/aggregation (26MB AllGather instead of
52MB); metadata tables fed pre-transposed from host (contiguous DMA); xp and
h1 SBUF-resident (no per-window reloads); h1 DRAM writes batched 4 windows
per DMA; deep tile pools for cross-window pipelining; bf16 pool exchange
with tightened mask width; single-DMA final output.

Compact-id space: for each (producer p, consumer c) pair the unique source
slots are ranked (sorted by slot); compact id = p*MAXH + rank.  The same id
space indexes xc (x features, layer 1) and h_compact (stage-A output,
layer 2), so one edge-index table serves both layers.
"""

import math
from contextlib import ExitStack

import numpy as np

P = 128
IN_DIM = 128
HID = 256
BF16 = True  # bf16 matmul operands / gather tables (PSUM accum stays fp32)
EDT_FP8 = True   # h1 exchange (AllGather + stage A + L2 gather) in fp8e4
XC_FP8 = False   # xc table + L1 edge gather in fp8e4
POOL_BF16 = True  # pool partial exchange in bf16


def XDT_MYBIR():
    import concourse.mybir as mybir
    return mybir.dt.float8e4 if XC_FP8 else (
        mybir.dt.bfloat16 if BF16 else mybir.dt.float32)
ABLATE = set()  # timing ablations: {"h1ag","poolag","stagea","l1g","l2g",
# "aggmm","trmm","dve","h1w"}


def EDT_MYBIR():
    import concourse.mybir as mybir
    return mybir.dt.float8e4 if EDT_FP8 else (
        mybir.dt.bfloat16 if BF16 else mybir.dt.float32)
SACALL = 1024  # max gather call size (64 desc/engine packet limit)
N_CORES = 8


def _sa_calls(SAK):
    """Call plan: sizes (each %128==0, <=1024) with offsets."""
    plan, off = [], 0
    while off < SAK:
        L = min(SACALL, SAK - off)
        plan.append((off, L))
        off += L
    return plan


# ----------------------------------------------------------------- host prep


def _pack(node_ids, degs, n_bins):
    """Least-loaded-first packing into n_bins bins of <=128 nodes, balancing
    sum of degs.  Returns (win, pos, max_load)."""
    import heapq

    order = np.argsort(-degs, kind="stable")
    nb = len(node_ids)
    win = np.empty(nb, np.int32)
    pos = np.empty(nb, np.int32)
    counts = np.zeros(n_bins, np.int32)
    loads = np.zeros(n_bins, np.int64)
    heap = [(0, b) for b in range(n_bins)]
    heapq.heapify(heap)
    for oi in order:
        while True:
            load, b = heapq.heappop(heap)
            if counts[b] < P:
                break
        win[oi] = b
        pos[oi] = counts[b]
        counts[b] += 1
        loads[b] = load + degs[oi]
        heapq.heappush(heap, (loads[b], b))
    return win, pos, loads.max()


def wrap16_plan(vals, plan):
    """Layout int16 idx array for dma_gather: per call segment of size L,
    idx j lands at [j%16, col0 + j//16].  Replicated across 8 gpsimd cores."""
    v = np.asarray(vals, np.int16)
    segs = []
    per = plan[-1][0] + plan[-1][1]
    n = len(v) // per
    for r in range(n):
        for off, L in plan:
            seg = v[r * per + off : r * per + off + L]
            segs.append(seg.reshape(L // 16, 16).T)
    out = np.concatenate(segs, axis=1)
    return np.tile(out, (8, 1))


def _prep(x, edge_index, batch, n_graphs, n_cores):
    N = x.shape[0]
    E = edge_index.shape[1]
    NPC = N // n_cores

    src = np.asarray(edge_index[0], np.int64)
    dst = np.asarray(edge_index[1], np.int64)
    batch = np.asarray(batch, np.int64)

    deg = (np.bincount(dst, minlength=N) + 1).astype(np.float32)
    dinv = (1.0 / np.sqrt(deg)).astype(np.float32)
    norm_e = (dinv[src] * dinv[dst]).astype(np.float32)
    selfw = (dinv * dinv).astype(np.float32)
    cnt = np.bincount(batch, minlength=n_graphs).astype(np.float32)
    icnt_g = (1.0 / np.maximum(cnt, 1.0)).astype(np.float32)

    edge_core = dst // NPC  # consumer (dst owner)
    src_core = src // NPC  # producer (src owner)

    # pass 1: per-core window packing of own nodes.
    indeg = np.bincount(dst, minlength=N)
    W_base = math.ceil(NPC / P)
    chosen = None
    for K_try, extra in [(3, 0), (3, 2), (3, 4), (3, 6), (4, 0)]:
        W_try = W_base + extra
        cap = K_try * P
        packs = []
        ok = True
        for c in range(n_cores):
            ids = np.arange(c * NPC, (c + 1) * NPC)
            w, p, mx = _pack(ids, indeg[ids].astype(np.int64), W_try)
            if mx > cap:
                ok = False
                break
            packs.append((w, p))
        if ok:
            chosen = (K_try, W_try, packs)
            break
    assert chosen is not None
    K, W, packs = chosen
    NSLOT = W * P
    win_of = np.empty(N, np.int32)
    pos_of = np.empty(N, np.int32)
    for c in range(n_cores):
        ids = np.arange(c * NPC, (c + 1) * NPC)
        w, p = packs[c]
        win_of[ids] = w
        pos_of[ids] = p
    slot_of = win_of.astype(np.int64) * P + pos_of  # local slot within owner
    NCHUNK = W * K
    ES = NCHUNK * P

    # slot -> node id per core
    inv_slot = np.full((n_cores, NSLOT), -1, np.int64)
    for c in range(n_cores):
        ids = np.arange(c * NPC, (c + 1) * NPC)
        inv_slot[c, slot_of[ids]] = ids

    # unique src slots per (producer, consumer) pair
    u_pc = [[None] * n_cores for _ in range(n_cores)]  # [p][c] -> slots
    maxcnt = 0
    for c in range(n_cores):
        m = edge_core == c
        for p in range(n_cores):
            mp = m & (src_core == p)
            u = np.unique(slot_of[src[mp]])
            u_pc[p][c] = u
            maxcnt = max(maxcnt, len(u))
    MAXH = int(np.ceil(maxcnt / P)) * P
    UC = n_cores * MAXH
    assert UC <= 32768, f"compact table too large: {UC} (maxcnt {maxcnt})"

    # gather batches are capped at 1024 idxs
    NBW = 1
    for cand in (4, 2):
        if W % cand == 0 and cand * K * 128 <= 1024:
            NBW = cand
            break
    if NBW == 1 and K * 128 > 1024:
        raise AssertionError(f"K={K} too large for single gather batch")

    GBLK = n_graphs // n_cores  # graphs per core for ReduceScatter

    per_core = []
    import ml_dtypes

    cdt = ml_dtypes.bfloat16 if BF16 else np.float32
    xnp = np.asarray(x)

    for c in range(n_cores):
        m = np.flatnonzero(edge_core == c)
        e_src, e_dst, e_norm = src[m], dst[m], norm_e[m]
        e_win = win_of[e_dst]
        order = np.argsort(e_win, kind="stable")
        e_src, e_dst, e_norm, e_win = (
            e_src[order],
            e_dst[order],
            e_norm[order],
            e_win[order],
        )
        wc = np.bincount(e_win, minlength=W)
        assert wc.max() <= K * P

        # compact id per edge source: p*MAXH + rank in u_pc[p][c]
        ep = e_src // NPC
        cids = np.empty(len(e_src), np.int64)
        for p in range(n_cores):
            mk = ep == p
            cids[mk] = p * MAXH + np.searchsorted(
                u_pc[p][c], slot_of[e_src[mk]]
            )

        # xc table: x features in compact layout (for layer 1)
        import ml_dtypes as _md
        xdt_np = _md.float8_e4m3 if XC_FP8 else cdt
        xc = np.zeros((UC, IN_DIM), xdt_np)
        for p in range(n_cores):
            u = u_pc[p][c]
            xc[p * MAXH : p * MAXH + len(u)] = xnp[inv_slot[p, u]]

        # stage-A index table (this core as consumer): for each producer p,
        # local slots of u_pc[p][c] within p's h_full block, padded with 0
        # (gathers row 0 into unused compact rows; negative "skip" indices
        # crash the DGE on this stack).
        sidx = np.zeros(UC, np.int64)
        for p in range(n_cores):
            u = u_pc[p][c]
            sidx[p * MAXH : p * MAXH + len(u)] = u

        # per-window edge slots
        gidx_slots = np.zeros(ES, np.int64)
        dstloc = np.zeros(ES, np.float32)
        nrm = np.zeros(ES, np.float32)
        off = np.concatenate([[0], np.cumsum(wc)])
        for w in range(W):
            a, b = off[w], off[w + 1]
            sl = w * K * P
            nw = b - a
            gidx_slots[sl : sl + nw] = cids[a:b]
            dstloc[sl : sl + nw] = pos_of[e_dst[a:b]]
            nrm[sl : sl + nw] = e_norm[a:b]

        # dn table pre-transposed host-side: [P, 2*NCHUNK]
        dn = np.concatenate(
            [dstloc.reshape(NCHUNK, P).T, nrm.reshape(NCHUNK, P).T], axis=1
        ).astype(np.float32)

        # per-slot node metadata (window-permuted own nodes), [P, W] layouts
        ids = np.arange(c * NPC, (c + 1) * NPC)
        slot_node = np.full(NSLOT, -1, np.int64)
        slot_node[slot_of[ids]] = ids
        valid = slot_node >= 0
        dsq = np.zeros(NSLOT, np.float32)
        dsq[valid] = selfw[slot_node[valid]]
        xp = np.zeros((NSLOT, IN_DIM), cdt)
        xp[valid] = xnp[slot_node[valid]]
        xp = np.ascontiguousarray(
            xp.reshape(W, P, IN_DIM).transpose(1, 0, 2).reshape(P, W * IN_DIM))
        bvals = np.full(NSLOT, -1.0, np.float32)
        icn = np.zeros(NSLOT, np.float32)
        gb_c = int(batch[c * NPC])
        bvals[valid] = (batch[slot_node[valid]] - gb_c).astype(np.float32)
        icn[valid] = icnt_g[batch[slot_node[valid]]]

        per_core.append(
            dict(
                xc=xc,
                xp=xp,
                _gidx_slots=gidx_slots,
                dn=dn,
                dsq=dsq.reshape(W, P).T.astype(np.float32),
                bloc=bvals.reshape(W, P).T.astype(np.float32),
                icnt=icn.reshape(W, P).T.astype(np.float32),
                sidx=wrap16_plan(sidx, _sa_calls(MAXH)),
                gb=gb_c,
            )
        )

    gbs = [pc["gb"] for pc in per_core]
    gspan = max(
        int(batch[(c + 1) * NPC - 1]) - gbs[c] + 1 for c in range(n_cores)
    )
    GW = int(np.ceil(gspan / 16)) * 16
    assert GW <= 512, GW

    meta = dict(
        N=N, E=E, NPC=NPC, W=W, NSLOT=NSLOT, K=K, NCHUNK=NCHUNK, ES=ES,
        MAXH=MAXH, UC=UC, GW=GW, gbs=gbs, n_graphs=n_graphs,
        n_cores=n_cores, NBW=NBW, GBLK=GBLK,
    )
    return per_core, meta


# ------------------------------------------------------------- device kernel


def _build_program(meta):
    import concourse.bacc as bacc
    import concourse.bass as bass
    import concourse.mybir as mybir
    import concourse.tile as tile

    f32 = mybir.dt.float32
    cdt = mybir.dt.bfloat16 if BF16 else mybir.dt.float32
    edt = EDT_MYBIR()
    xdt = XDT_MYBIR()
    pdt = mybir.dt.bfloat16 if POOL_BF16 else f32
    i16 = mybir.dt.int16
    i32 = mybir.dt.int32
    Alu = mybir.AluOpType
    Act = mybir.ActivationFunctionType

    W, K, ES, NSLOT = meta["W"], meta["K"], meta["ES"], meta["NSLOT"]
    NCHUNK, MAXH, UC, GW = meta["NCHUNK"], meta["MAXH"], meta["UC"], meta["GW"]
    gbs = meta["gbs"]
    G = meta["n_graphs"]
    GBLK = meta["GBLK"]
    n_cores = meta["n_cores"]
    GLOBW = max(G, max(gbs) + GW)
    NBW = meta["NBW"]  # windows per gather batch
    EB = NBW * K * P  # edge slots per gather batch
    HB = 4  # windows per h1_local write batch

    nc = bacc.Bacc(None, target_bir_lowering=False)

    ext_in = {}
    for name, shape, dt in [
        ("xc", [UC, IN_DIM], xdt),
        ("xp", [P, W * IN_DIM], cdt),
        ("gidx", [P, ES // 16], i16),
        ("dn", [P, 2 * NCHUNK], f32),
        ("dsq", [P, W], f32),
        ("bloc", [P, W], f32),
        ("icnt", [P, W], f32),
        ("sidx", [P, UC // 16], i16),
        ("w1", [IN_DIM, HID], cdt),
        ("w2", [HID, HID], cdt),
        ("wf", [HID, 1], f32),
        ("b1", [1, HID], cdt),
        ("b2", [1, HID], cdt),
        ("bf", [1, 1], f32),
    ]:
        ext_in[name] = nc.dram_tensor(name, shape, dt, kind="ExternalInput")
    out_ext = nc.dram_tensor("out", [G, 1], f32, kind="ExternalOutput")

    h1_local = nc.dram_tensor("h1_local", [NSLOT, HID], edt)
    h_full = nc.dram_tensor("h_full", [NSLOT * n_cores, HID], edt,
                            addr_space="Shared")
    h_compact = nc.dram_tensor("h_compact", [UC, HID], edt)
    pool_part = nc.dram_tensor("pool_part", [HID, GW], pdt)
    pool_all = nc.dram_tensor("pool_all", [HID * n_cores, GW], pdt,
                              addr_space="Shared")

    core_ids = list(range(n_cores))

    with ExitStack() as ctx:
        tc = ctx.enter_context(tile.TileContext(nc, num_cores=n_cores))
        cst = ctx.enter_context(tc.tile_pool(name="cst", bufs=1))
        sbw = ctx.enter_context(tc.tile_pool(name="sbw", bufs=10))
        xgp = ctx.enter_context(tc.tile_pool(name="xgp", bufs=4))
        hp = ctx.enter_context(tc.tile_pool(name="hp", bufs=6))
        ps_agg = ctx.enter_context(
            tc.tile_pool(name="ps_agg", bufs=4, space="PSUM"))
        ps_tr = ctx.enter_context(
            tc.tile_pool(name="ps_tr", bufs=2, space="PSUM"))
        ps_pool = ctx.enter_context(
            tc.tile_pool(name="ps_pool", bufs=1, space="PSUM"))

        # ---- constants / metadata loads (all contiguous layouts)
        gidx_t = cst.tile([P, ES // 16], i16)
        nc.sync.dma_start(out=gidx_t[:], in_=ext_in["gidx"][:, :])
        sidx_t = cst.tile([P, UC // 16], i16)
        nc.sync.dma_start(out=sidx_t[:], in_=ext_in["sidx"][:, :])
        dn_t = cst.tile([P, 2 * NCHUNK], f32)
        nc.sync.dma_start(out=dn_t[:], in_=ext_in["dn"][:, :])
        dsq_t = cst.tile([P, W], f32)
        nc.sync.dma_start(out=dsq_t[:], in_=ext_in["dsq"][:, :])
        bloc_t = cst.tile([P, W], f32)
        nc.sync.dma_start(out=bloc_t[:], in_=ext_in["bloc"][:, :])
        icnt_t = cst.tile([P, W], f32)
        nc.sync.dma_start(out=icnt_t[:], in_=ext_in["icnt"][:, :])
        w1_t = cst.tile([IN_DIM, HID], cdt)
        nc.sync.dma_start(out=w1_t[:], in_=ext_in["w1"][:, :])
        w2_t = cst.tile([P, 2 * HID], cdt)  # W2 K-halves side by side
        nc.sync.dma_start(
            out=w2_t[:].rearrange("k (s h) -> k s h", s=2),
            in_=ext_in["w2"].rearrange("(s k) h -> k s h", k=P))
        wf_t = cst.tile([P, 2], f32)  # wf halves: [:, 0], [:, 1]
        nc.sync.dma_start(
            out=wf_t[:].rearrange("k (s o) -> k s o", s=2),
            in_=ext_in["wf"].rearrange("(s k) o -> k s o", k=P))
        b1_t = cst.tile([1, HID], cdt)
        nc.sync.dma_start(out=b1_t[:], in_=ext_in["b1"][:, :])
        b2_t = cst.tile([1, HID], cdt)
        nc.sync.dma_start(out=b2_t[:], in_=ext_in["b2"][:, :])
        bf_t = cst.tile([1, 1], f32)
        nc.sync.dma_start(out=bf_t[:], in_=ext_in["bf"][:, :])

        # xp resident in SBUF: [P, W, IN_DIM] (host feeds pre-transposed)
        xp_all = cst.tile([P, W, IN_DIM], cdt)
        nc.sync.dma_start(
            out=xp_all[:],
            in_=ext_in["xp"].rearrange("p (w d) -> p w d", d=IN_DIM))

        # h1 resident in SBUF (written by L1, read by L2 self-loop);
        # kept in the exchange dtype so DRAM writes need no cast.
        h1_all = cst.tile([P, W, HID], edt)

        ones_t = cst.tile([1, P], cdt)
        nc.vector.memset(ones_t[:], 1.0)
        ones_f = cst.tile([1, P], f32)
        nc.vector.memset(ones_f[:], 1.0)

        iota_i = cst.tile([P, K * P], i32)
        nc.gpsimd.iota(iota_i[:], pattern=[[0, K], [1, P]], base=0,
                       channel_multiplier=0)
        iota_f = cst.tile([P, K * P], f32)
        nc.vector.tensor_copy(out=iota_f[:], in_=iota_i[:])
        iotag_i = cst.tile([P, GW], i32)
        nc.gpsimd.iota(iotag_i[:], pattern=[[1, GW]], base=0,
                       channel_multiplier=0)
        iotag_f = cst.tile([P, GW], f32)
        nc.vector.tensor_copy(out=iotag_f[:], in_=iotag_i[:])
        pcol_i = cst.tile([P, 1], i32)
        nc.gpsimd.iota(pcol_i[:], pattern=[[0, 1]], base=0,
                       channel_multiplier=1)
        pcol_f = cst.tile([P, 1], f32)
        nc.vector.tensor_copy(out=pcol_f[:], in_=pcol_i[:])

        # ablation stand-ins: single constant tiles replacing per-window
        # producers, so consumers keep a valid producer without the real op.
        sconst = None
        if "dve" in ABLATE:
            sconst = cst.tile([P, P], cdt)
            nc.vector.memset(sconst[:], 0.0)
            sconst_g = cst.tile([P, GW], edt)
            nc.vector.memset(sconst_g[:], 0.0)
        xg_const = None
        if "l1g" in ABLATE:
            xg_const = cst.tile([P, NBW * K, IN_DIM], xdt)
            nc.vector.memset(xg_const[:], 0.0)
        hg_const = None
        if "l2g" in ABLATE:
            hg_const = cst.tile([P, NBW * K, HID], edt)
            nc.vector.memset(hg_const[:], 0.0)

        # ---------------- layer 1
        next_w = 0
        for b in range(W // NBW):
            if "l1g" in ABLATE:
                xg = xg_const
            else:
                xg = xgp.tile([P, NBW * K, IN_DIM], xdt, tag="xg")
                nc.gpsimd.dma_gather(
                    out_ap=xg[:],
                    in_ap=ext_in["xc"][:, :],
                    idxs_ap=gidx_t[:, b * (EB // 16) : (b + 1) * (EB // 16)],
                    num_idxs=EB,
                    num_idxs_reg=EB,
                    elem_size=IN_DIM,
                )
            for wl in range(NBW):
                w = b * NBW + wl
                aggp = ps_agg.tile([P, P], f32, space="PSUM", tag="aggp")
                if "aggmm" not in ABLATE:
                    for ck in range(K):
                        if "dve" in ABLATE:
                            s_t = sconst
                        else:
                            s_t = sbw.tile([P, P], cdt, tag="s_t")
                            cg = w * K + ck
                            nc.vector.tensor_scalar(
                                out=s_t[:],
                                in0=iota_f[:, ck * P : (ck + 1) * P],
                                scalar1=dn_t[:, cg : cg + 1],
                                scalar2=dn_t[:, NCHUNK + cg : NCHUNK + cg + 1],
                                op0=Alu.is_equal,
                                op1=Alu.mult,
                            )
                        nc.tensor.matmul(
                            out=aggp[:],
                            lhsT=xg[:, wl * K + ck, :],
                            rhs=s_t[:],
                            start=(ck == 0),
                            stop=False,
                        )
                # self loops: aggT += xp_w^T @ diag(dsq_w)
                if "dve" in ABLATE:
                    diag = sconst
                else:
                    diag = sbw.tile([P, P], cdt, tag="diag")
                    nc.vector.tensor_scalar(
                        out=diag[:],
                        in0=iota_f[:, 0:P],
                        scalar1=pcol_f[:, 0:1],
                        scalar2=dsq_t[:, w : w + 1],
                        op0=Alu.is_equal,
                        op1=Alu.mult,
                    )
                nc.tensor.matmul(
                    out=aggp[:], lhsT=xp_all[:, w, :], rhs=diag[:],
                    start=("aggmm" in ABLATE), stop=True)
                agg_sb = sbw.tile([P, P], cdt, tag="agg_sb")
                nc.scalar.copy(out=agg_sb[:], in_=aggp[:])
                # transform: h1 = relu(aggT.T @ W1 + b1)
                hpsum = ps_tr.tile([P, HID], f32, space="PSUM", tag="hpsum")
                if "trmm" not in ABLATE:
                    nc.tensor.matmul(
                        out=hpsum[:], lhsT=agg_sb[:], rhs=w1_t[:],
                        start=True, stop=False)
                nc.tensor.matmul(
                    out=hpsum[:], lhsT=ones_t[:], rhs=b1_t[:],
                    start=("trmm" in ABLATE), stop=True)
                nc.scalar.activation(
                    out=h1_all[:, w, :], in_=hpsum[:], func=Act.Relu)
            # batched h1_local writes every HB windows (tail handled)
            done = (b + 1) * NBW
            while next_w + HB <= done or (done == W and next_w < W):
                nb_ = min(HB, W - next_w)
                if "h1w" not in ABLATE:
                    nc.sync.dma_start(
                        out=h1_local[next_w * P : (next_w + nb_) * P, :]
                        .rearrange("(c p) d -> p c d", p=P),
                        in_=h1_all[:, next_w : next_w + nb_, :])
                next_w += nb_

        # ---------------- AllGather h1, then per-producer compaction
        if "h1ag" not in ABLATE:
            nc.gpsimd.collective_compute(
                "AllGather", Alu.bypass, replica_groups=[core_ids],
                ins=[h1_local[:, :]], outs=[h_full[:, :]])
        sa_plan = [] if "stagea" in ABLATE else _sa_calls(MAXH)
        for p in range(n_cores):
            for off, L in sa_plan:
                hc = xgp.tile([P, SACALL // P, HID], edt, tag="hc")
                cbase = p * MAXH + off
                icol = cbase // 16
                nc.gpsimd.dma_gather(
                    out_ap=hc[:, : L // P, :],
                    in_ap=h_full[p * NSLOT : (p + 1) * NSLOT, :],
                    idxs_ap=sidx_t[:, icol : icol + L // 16],
                    num_idxs=L,
                    num_idxs_reg=L,
                    elem_size=HID,
                )
                nc.sync.dma_start(
                    out=h_compact[cbase : cbase + L, :].rearrange(
                        "(c p) d -> p c d", p=P),
                    in_=hc[:, : L // P, :])

        # ---------------- layer 2 + pooling
        # glob: [2][P, GLOBW] f32 pooled partial sums at global graph columns
        glob = []
        for h in range(2):
            gt = cst.tile([P, GLOBW], f32, tag=f"glob{h}")
            nc.vector.memset(gt[:], 0.0)
            glob.append(gt)
        poolp = []
        for h in range(2):
            pt = ps_pool.tile([P, GW], f32, space="PSUM", tag=f"poolp{h}")
            poolp.append(pt)
        for b in range(W // NBW):
            if "l2g" in ABLATE:
                hg = hg_const
            else:
                hg = xgp.tile([P, NBW * K, HID], edt, tag="hg")
                nc.gpsimd.dma_gather(
                    out_ap=hg[:],
                    in_ap=h_compact[:, :],
                    idxs_ap=gidx_t[:, b * (EB // 16) : (b + 1) * (EB // 16)],
                    num_idxs=EB,
                    num_idxs_reg=EB,
                    elem_size=HID,
                )
            for wl in range(NBW):
                w = b * NBW + wl
                aggp2 = []
                for h in range(2):
                    a2t = ps_agg.tile([P, P], f32, space="PSUM", tag="aggp")
                    aggp2.append(a2t)
                if "aggmm" not in ABLATE:
                    for ck in range(K):
                        if "dve" in ABLATE:
                            s_t = sconst_g
                        else:
                            s_t = sbw.tile([P, P], edt, tag="s_t2")
                            cg = w * K + ck
                            nc.vector.tensor_scalar(
                                out=s_t[:],
                                in0=iota_f[:, ck * P : (ck + 1) * P],
                                scalar1=dn_t[:, cg : cg + 1],
                                scalar2=dn_t[:, NCHUNK + cg : NCHUNK + cg + 1],
                                op0=Alu.is_equal,
                                op1=Alu.mult,
                            )
                        for h in range(2):
                            nc.tensor.matmul(
                                out=aggp2[h][:],
                                lhsT=hg[:, wl * K + ck, h * P : (h + 1) * P],
                                rhs=s_t[:, 0:P],
                                start=(ck == 0),
                                stop=False,
                            )
                if "dve" in ABLATE:
                    diag = sconst_g
                else:
                    diag = sbw.tile([P, P], edt, tag="diag2")
                    nc.vector.tensor_scalar(
                        out=diag[:],
                        in0=iota_f[:, 0:P],
                        scalar1=pcol_f[:, 0:1],
                        scalar2=dsq_t[:, w : w + 1],
                        op0=Alu.is_equal,
                        op1=Alu.mult,
                    )
                for h in range(2):
                    nc.tensor.matmul(
                        out=aggp2[h][:],
                        lhsT=h1_all[:, w, h * P : (h + 1) * P],
                        rhs=diag[:, 0:P],
                        start=("aggmm" in ABLATE),
                        stop=True,
                    )
                hpsum = ps_tr.tile([P, HID], f32, space="PSUM", tag="hpsum")
                if "trmm" not in ABLATE:
                    for h in range(2):
                        agg_sb = sbw.tile([P, P], cdt, tag=f"agg2sb{h}")
                        nc.scalar.copy(out=agg_sb[:], in_=aggp2[h][:])
                        nc.tensor.matmul(
                            out=hpsum[:],
                            lhsT=agg_sb[:],
                            rhs=w2_t[:, h * HID : (h + 1) * HID],
                            start=(h == 0),
                            stop=False,
                        )
                nc.tensor.matmul(
                    out=hpsum[:], lhsT=ones_t[:], rhs=b2_t[:],
                    start=("trmm" in ABLATE), stop=True)
                # h2s = relu(h2) * icnt
                h2s = hp.tile([P, HID], edt, tag="h2s")
                nc.scalar.activation(
                    out=h2s[:], in_=hpsum[:], func=Act.Relu,
                    scale=icnt_t[:, w : w + 1])
                # pooling mask + matmuls
                if "dve" in ABLATE:
                    mask = sconst_g
                else:
                    mask = sbw.tile([P, GW], edt, tag="mask")
                    nc.vector.tensor_scalar(
                        out=mask[:], in0=iotag_f[:],
                        scalar1=bloc_t[:, w : w + 1],
                        scalar2=None, op0=Alu.is_equal)
                for h in range(2):
                    nc.tensor.matmul(
                        out=poolp[h][:],
                        lhsT=h2s[:, h * P : (h + 1) * P],
                        rhs=mask[:],
                        start=(w == 0),
                        stop=(w == W - 1),
                    )

        # ---------------- pooled shards out + AllGather
        for h in range(2):
            pp = hp.tile([P, GW], pdt, tag="ppsb")
            nc.scalar.copy(out=pp[:], in_=poolp[h][:])
            nc.sync.dma_start(
                out=pool_part[h * P : (h + 1) * P, :], in_=pp[:])
        if "poolag" not in ABLATE:
            nc.gpsimd.collective_compute(
                "AllGather", Alu.bypass, replica_groups=[core_ids],
                ins=[pool_part[:, :]], outs=[pool_all[:, :]])

        # ---------------- combine shards into global pooledT
        NBLK = math.ceil(G / P)
        for r in range(n_cores):
            for h in range(2):
                sh = hp.tile([P, GW], pdt, tag="sh")
                nc.sync.dma_start(
                    out=sh[:],
                    in_=(pool_part[h * P : (h + 1) * P, :]
                         if "poolag" in ABLATE else
                         pool_all[r * HID + h * P : r * HID + (h + 1) * P, :]))
                nc.vector.tensor_tensor(
                    out=glob[h][:, gbs[r] : gbs[r] + GW],
                    in0=glob[h][:, gbs[r] : gbs[r] + GW],
                    in1=sh[:],
                    op=Alu.add,
                )

        # ---------------- final linear + sigmoid
        out_sb = cst.tile([P, NBLK], f32, tag="out_sb")
        for bk in range(NBLK):
            lp = ps_tr.tile([P, 1], f32, space="PSUM", tag="hpsum")
            for h in range(2):
                nc.tensor.matmul(
                    out=lp[:],
                    lhsT=glob[h][:, bk * P : (bk + 1) * P],
                    rhs=wf_t[:, h : h + 1],
                    start=(h == 0),
                    stop=False,
                )
            nc.tensor.matmul(
                out=lp[:], lhsT=ones_f[:], rhs=bf_t[:],
                start=False, stop=True)
            nc.scalar.activation(
                out=out_sb[:, bk : bk + 1], in_=lp[:], func=Act.Sigmoid)
        assert G == NBLK * P
        nc.sync.dma_start(
            out=out_ext.rearrange("(b p) o -> p (b o)", p=P),
            in_=out_sb[:, :NBLK])

    nc.compile()
    return nc, meta


def _layout_gidx(gidx_slots, meta):
    """edge-slot compact ids -> int16 wrapped layout for batched dma_gather."""
    K = meta["K"]
    EB = meta["NBW"] * K * P
    v = np.asarray(gidx_slots, np.int16).reshape(-1, EB)
    cw = EB // 16
    out = np.empty((16, v.size // 16), np.int16)
    for b in range(v.shape[0]):
        out[:, b * cw : (b + 1) * cw] = v[b].reshape(cw, 16).T
    return np.tile(out, (8, 1))


def kernel(x, edge_index, batch, W1, b1, W2, b2, Wf, bf,
           n_graphs=2048, n_cores=8):
    from concourse.bass_utils import run_bass_kernel_spmd

    x = np.asarray(x, np.float32)
    per_core, meta = _prep(x, edge_index, batch, n_graphs, n_cores)
    nc, meta = _build_program(meta)

    import ml_dtypes
    cdt = ml_dtypes.bfloat16 if BF16 else np.float32
    w_comm = dict(
        w1=np.asarray(W1, np.float32).astype(cdt),
        w2=np.asarray(W2, np.float32).astype(cdt),
        wf=np.asarray(Wf, np.float32).reshape(HID, 1),
        b1=np.asarray(b1, np.float32).reshape(1, HID).astype(cdt),
        b2=np.asarray(b2, np.float32).reshape(1, HID).astype(cdt),
        bf=np.asarray(bf, np.float32).reshape(1, 1),
    )
    in_maps = []
    for pc in per_core:
        m = dict(w_comm)
        m["xc"] = pc["xc"]
        m["xp"] = pc["xp"]
        m["gidx"] = _layout_gidx(pc["_gidx_slots"], meta)
        m["dn"] = pc["dn"]
        m["dsq"] = pc["dsq"]
        m["bloc"] = pc["bloc"]
        m["icnt"] = pc["icnt"]
        m["sidx"] = pc["sidx"]
        in_maps.append(m)

    res = run_bass_kernel_spmd(nc, in_maps, list(range(n_cores)))
    return np.asarray(res.results[0]["out"], np.float32)

